# revision 11
# baseline (speedup 1.0000x reference)
"""FALCON ObjectSomeValuesFrom forward kernel for Trainium2 (Bass/Tile).

Math: the reference computes
    c_fs[j]   = sigmoid(cw + col_j + b)
    r_fs[i,j] = sigmoid(row_i + col_j + b)
    out[i]    = max_j r_fs[i,j] * c_fs[j]
with col_j = e_j . w_r, row_i = e_i . w_l + rw, cw = c_emb . w_l,
rw = r_emb . w_l.  Both product factors are strictly increasing in col_j,
so the max over j is attained at argmax_j col_j for every i:
    out[i] = sigmoid(a_i + rw + colmax + b) * sigmoid(cw + colmax + b)
with a_i = e_i . w_l and colmax = max_j col_j.  The O(N^2) pairwise block
collapses to two GEMVs over e_all plus an elementwise sigmoid tail.

Implementation: the table is pre-transposed on the host to eT [128, 8192]
(feature dim on partitions) and quantized to fp8-e3m4 with a power-of-two
scale.  Both GEMVs then run on the tensor engine as 64 self-loading
matmuls (stationary = eT 128x128 block, moving = [w_l, w_r] as 2 fp8
columns), which fuses the multiply and the d-reduction and leaves the
vector engine free.  Per-DMA-chunk strided reduce_max passes + a gpsimd
partition_all_reduce produce colmax broadcast to all partitions, the
activation engine computes the sigmoid tail, and the result is written
back with a pre-prepared SWDGE kv_writeback fired by trigger_dma (which
skips the descriptor-generation latency of a plain DMA on the critical
path).  Every core runs the identical program on the identical full
inputs (colmax needs every row, and the modeled collective cost is far
larger than replicating the scan), so core 0's output is the full answer.
"""

import numpy as np

N = 8192        # 8000 named + 192 anon entities
D = 128         # emb dim
NCORES = 8
RPC = N // NCORES    # kept for test.py compatibility
P = 128              # SBUF partitions
NBLK = N // P        # 64 matmul blocks of 128 rows
DMA_CHUNKS = 4
BPC = NBLK // DMA_CHUNKS   # matmul blocks per DMA chunk
SE = 4.0             # host scale on e before fp8 quantization
SW = 4.0             # host scale on w before fp8 quantization
COL_DT = "fp8"       # table precision: "fp8" (e3m4) or "fp16"

_CACHE = {}


def _build_nc(repeat=1, col_dt=COL_DT):
    import concourse.bass as bass
    import concourse.bacc as bacc
    import concourse.tile as tile
    import concourse.mybir as mybir
    from concourse import bass_isa

    f32 = mybir.dt.float32
    i32 = mybir.dt.int32
    u8 = mybir.dt.uint8
    tdt = {"fp8": mybir.dt.float8e3, "fp16": mybir.dt.float16}[col_dt]
    inv_s = (1.0 / (SE * SW)) if col_dt == "fp8" else 1.0

    nc = bacc.Bacc("TRN2", target_bir_lowering=False, debug=False)

    eTd = nc.dram_tensor("eT", [P, N], tdt, kind="ExternalInput").ap()
    # aux: bytes 0-1 = [w_l, w_r] in table dtype (fp8), 4-11 = consts f32
    auxd = nc.dram_tensor("aux", [P, 12], u8, kind="ExternalInput").ap()
    out = nc.dram_tensor("out", [N], f32, kind="ExternalOutput").ap()

    with tile.TileContext(nc) as tc:
        with (
            tc.tile_pool(name="sb", bufs=1) as sb,
            tc.tile_pool(name="ps", bufs=1, space=bass.MemorySpace.PSUM) as ps,
        ):
            aux_t = sb.tile([P, 12], u8)
            nc.gpsimd.dma_start(aux_t[:], auxd)
            wc_t = aux_t[:, 0:4].bitcast(tdt)       # [P, 4/tdt-size], cols 0:2 used
            consts_t = aux_t[:, 4:12].bitcast(f32)  # [P, 2]

            # Dummy sigmoid so the activation table load is scheduled early,
            # overlapping the table DMA instead of sitting on the tail.
            scr = sb.tile([P, 1], f32)
            nc.vector.memset(scr[:], 0.0)
            scr2 = sb.tile([P, 1], f32)
            nc.scalar.activation(scr2[:], scr[:], mybir.ActivationFunctionType.Sigmoid)

            # Writeback indices for the prepared kv_writeback (all zeros).
            idxs = sb.tile([P, 1], i32)
            nc.vector.memset(idxs[:], 0)

            et = sb.tile([P, N], tdt)
            step = N // DMA_CHUNKS
            for c in range(DMA_CHUNKS):
                nc.sync.dma_start(
                    et[:, c * step : (c + 1) * step],
                    eTd[:, c * step : (c + 1) * step],
                )

            # pt[p, 2b + t]: t=0 -> a_{128b+p} (w_l GEMV), t=1 -> col_{128b+p}
            pt = ps.tile([P, 2 * NBLK], f32)
            pt3 = pt[:].rearrange("p (n two) -> p n two", two=2)
            cm = sb.tile([P, DMA_CHUNKS], f32)
            a_col = pt[:, 1:2]
            for c in range(DMA_CHUNKS):
                for b in range(c * BPC, (c + 1) * BPC):
                    nc.tensor.matmul(
                        pt3[:, b, :],
                        et[:, b * P : (b + 1) * P],
                        wc_t[:, 0:2],
                        start=True,
                        stop=True,
                    )
                # Partial max over this chunk's col entries (overlaps later DMA)
                colv_c = bass.AP(
                    a_col.tensor, a_col.offset + 2 * c * BPC, [a_col.ap[0], [2, BPC]]
                )
                nc.vector.reduce_max(cm[:, c : c + 1], colv_c, axis=mybir.AxisListType.X)

            colm = sb.tile([P, 1], f32)
            nc.vector.reduce_max(colm[:], cm[:], axis=mybir.AxisListType.X)
            colmax = sb.tile([P, 1], f32)
            nc.gpsimd.partition_all_reduce(
                colmax[:], colm[:], channels=P, reduce_op=bass_isa.ReduceOp.max
            )

            # k1 = colmax/s + (rw + b);  k2 = sigmoid(colmax/s + (cw + b))
            k1 = sb.tile([P, 1], f32)
            nc.vector.tensor_scalar(
                k1[:], colmax[:], inv_s, consts_t[:, 0:1],
                op0=mybir.AluOpType.mult, op1=mybir.AluOpType.add,
            )
            k2 = sb.tile([P, 1], f32)
            nc.scalar.activation(
                k2[:], colmax[:], mybir.ActivationFunctionType.Sigmoid,
                bias=consts_t[:, 1:2], scale=inv_s,
            )

            # out = sigmoid(a/s + k1) * k2
            a_row = pt[:, 0:1]
            rowv = bass.AP(a_row.tensor, a_row.offset, [a_row.ap[0], [2, NBLK]])
            so = sb.tile([P, NBLK], f32)
            nc.scalar.activation(
                so[:], rowv, mybir.ActivationFunctionType.Sigmoid,
                bias=k1[:, 0:1], scale=inv_s,
            )
            fo = sb.tile([P, NBLK], f32)
            nc.vector.tensor_scalar_mul(fo[:], so[:], k2[:, 0:1])

            # Prepared SWDGE writeback: out_dev[p*64 + n] = fo[p, n]; the
            # trigger inherits the data dependency on fo, so only
            # trigger+transfer+sem sit on the tail (no HWDGE/DGE latency).
            out4 = out.rearrange("(b dhi dho n) -> b dhi dho n", b=1, dhi=P, dho=1)
            fo4 = fo[:].rearrange("p (dho b n) -> p dho b n", dho=1, b=1)
            wb_sem = nc.alloc_semaphore("wb_dma")
            nc.gpsimd.kv_writeback(
                out4, fo4, idxs[:], prepare_only=True, sem=wb_sem
            )
            nc.gpsimd.trigger_dma(count=None)
            nc.gpsimd.wait_ge(wb_sem, 16)

    nc.compile()
    return nc


def patch_for_timeline_sim(nc):
    """Make the module simulable by the no_exec TimelineSim.

    Tile schedules the kv_writeback prep on a DMASW proc lane and the final
    drain waits on that lane's semaphore.  CoreSim and real hardware satisfy
    it through their internal SWDGE ring bookkeeping, but the no_exec
    TimelineSim only fires on_update semaphores, so the wait starves.
    Attach the lane increment to the explicit wait_ge(wb_sem) instruction:
    it only becomes runnable after the actual DMA-completion semaphore, so
    the modeled timing stays honest.  Call this only on a module that is
    done running on hardware/CoreSim.
    """
    import concourse.mybir as mybir

    fn = nc.m.functions[0]
    insts = [i for blk in fn.blocks for i in blk.instructions]
    updated = set()
    waited = {}
    wb_waiter = None
    for inst in insts:
        si = inst.sync_info
        if si is None:
            continue
        for u in si.on_update or []:
            updated.add(u.id)
        for w in si.on_wait or []:
            if (w.ant_name or "").startswith("DMASW"):
                waited[w.id] = (w.ant_name, w.wait_value)
            if w.ant_name == "wb_dma":
                wb_waiter = inst
    starved = {i: v for i, v in waited.items() if i not in updated}
    if starved:
        assert wb_waiter is not None, "wb_dma waiter not found for DMASW patch"
        si = wb_waiter.sync_info
        fixes = [
            mybir.SyncUpdate(
                sync_type="semaphore", id=sid, ant_name=name,
                update_mode="sem-add-imm", update_value=val, update_reg=None,
            )
            for sid, (name, val) in starved.items()
        ]
        si.on_update = fixes + list(si.on_update or [])
    return nc


def get_nc(repeat=1, col_dt=COL_DT):
    key = ("nc", repeat, col_dt)
    if key not in _CACHE:
        _CACHE[key] = _build_nc(repeat, col_dt)
    return _CACHE[key]


def prepare_in_maps(
    anon_e_emb, e_table, c_table, r_table, fc0_w, fc0_b, c_id, r_id, col_dt=COL_DT
):
    import ml_dtypes

    e_all = np.concatenate(
        [np.asarray(e_table, np.float32), np.asarray(anon_e_emb, np.float32)], 0
    )
    fc0_w = np.asarray(fc0_w, np.float32)
    w_l = fc0_w[0, :D]
    w_r = fc0_w[0, D:]
    b = np.float32(np.asarray(fc0_b, np.float32)[0])
    c_emb = np.asarray(c_table, np.float32)[int(c_id)]
    r_emb = np.asarray(r_table, np.float32)[int(r_id)]
    rw = np.float32(np.dot(r_emb, w_l))
    cw = np.float32(np.dot(c_emb, w_l))

    if col_dt == "fp8":
        ndt, se, sw = ml_dtypes.float8_e3m4, SE, SW
    else:
        ndt, se, sw = np.float16, 1.0, 1.0
    eT = np.ascontiguousarray((e_all.T * se).astype(ndt))          # [128, 8192]
    wc = np.ascontiguousarray(
        (np.stack([w_l, w_r], axis=1) * sw).astype(ndt)            # [128, 2]
    )
    aux = np.zeros((P, 12), np.uint8)
    aux[:, 0:2] = wc.view(np.uint8)
    consts = np.empty((P, 2), np.float32)
    consts[:, 0] = rw + b
    consts[:, 1] = cw + b
    aux[:, 4:12] = consts.view(np.uint8)

    in_map = {"eT": eT, "aux": aux}
    return [dict(in_map) for _ in range(NCORES)]


def unscramble(out_dev: np.ndarray) -> np.ndarray:
    """Device layout [p*NBLK + n] -> true row order [n*P + p]."""
    return np.ascontiguousarray(out_dev.reshape(P, NBLK).T.reshape(-1))


def run(inputs, trace=False, trace_kwargs=None, repeat=1, col_dt=COL_DT):
    from concourse.bass_utils import run_bass_kernel_spmd

    nc = get_nc(repeat, col_dt)
    in_maps = prepare_in_maps(**inputs, col_dt=col_dt)
    res = run_bass_kernel_spmd(
        nc,
        in_maps,
        core_ids=list(range(NCORES)),
        trace=trace,
        **(trace_kwargs or {}),
    )
    out = unscramble(np.asarray(res.results[0]["out"]))
    return out, res


def kernel(**inputs) -> np.ndarray:
    out, _ = run(inputs, trace=False)
    return out


# revision 16
# speedup vs baseline: 1.1437x; 1.1437x over previous
"""FALCON ObjectSomeValuesFrom forward kernel for Trainium2 (Bass/Tile).

Math: the reference computes
    c_fs[j]   = sigmoid(cw + col_j + b)
    r_fs[i,j] = sigmoid(row_i + col_j + b)
    out[i]    = max_j r_fs[i,j] * c_fs[j]
with col_j = e_j . w_r, row_i = e_i . w_l + rw, cw = c_emb . w_l,
rw = r_emb . w_l.  Both product factors are strictly increasing in col_j,
so the max over j is attained at argmax_j col_j for every i:
    out[i] = sigmoid(a_i + rw + colmax + b) * sigmoid(cw + colmax + b)
with a_i = e_i . w_l and colmax = max_j col_j.  The O(N^2) pairwise block
collapses to two GEMVs over e_all plus an elementwise sigmoid tail.

Implementation: the table is pre-transposed on the host to eT [128, 8192]
(feature dim on partitions) and quantized to fp8-e3m4 with a power-of-two
scale.  Both GEMVs then run on the tensor engine as 64 self-loading
matmuls (stationary = eT 128x128 block, moving = [w_l, w_r] as 2 fp8
columns), which fuses the multiply and the d-reduction and leaves the
vector engine free.  Per-DMA-chunk strided reduce_max passes + a gpsimd
partition_all_reduce produce colmax broadcast to all partitions, the
activation engine computes the sigmoid tail, and the result is written
back with a pre-prepared SWDGE kv_writeback fired by trigger_dma (which
skips the descriptor-generation latency of a plain DMA on the critical
path).  Every core runs the identical program on the identical full
inputs (colmax needs every row, and the modeled collective cost is far
larger than replicating the scan), so core 0's output is the full answer.
"""

import numpy as np

N = 8192        # 8000 named + 192 anon entities
D = 128         # emb dim
NCORES = 8
RPC = N // NCORES    # kept for test.py compatibility
P = 128              # SBUF partitions
NBLK = N // P        # 64 matmul blocks of 128 rows
DMA_CHUNKS = 4
BPC = NBLK // DMA_CHUNKS   # matmul blocks per DMA chunk
SE = 4.0             # host scale on e before fp8 quantization
SW = 4.0             # host scale on w before fp8 quantization
COL_DT = "fp8"       # table precision: "fp8" (e3m4) or "fp16"

_CACHE = {}


def _build_nc(repeat=1, col_dt=COL_DT):
    import concourse.bass as bass
    import concourse.bacc as bacc
    import concourse.tile as tile
    import concourse.mybir as mybir
    from concourse import bass_isa

    f32 = mybir.dt.float32
    i32 = mybir.dt.int32
    u8 = mybir.dt.uint8
    tdt = {"fp8": mybir.dt.float8e3, "fp16": mybir.dt.float16}[col_dt]
    inv_s = (1.0 / (SE * SW)) if col_dt == "fp8" else 1.0

    nc = bacc.Bacc("TRN2", target_bir_lowering=False, debug=False)

    eTd = nc.dram_tensor("eT", [P, N], tdt, kind="ExternalInput").ap()
    # aux: bytes 0-1 = [w_l, w_r] in table dtype (fp8), 4-11 = consts f32
    auxd = nc.dram_tensor("aux", [P, 12], u8, kind="ExternalInput").ap()
    out = nc.dram_tensor("out", [N], f32, kind="ExternalOutput").ap()

    with tile.TileContext(nc) as tc:
        with (
            tc.tile_pool(name="sb", bufs=1) as sb,
            tc.tile_pool(name="ps", bufs=1, space=bass.MemorySpace.PSUM) as ps,
        ):
            aux_t = sb.tile([P, 12], u8)
            nc.gpsimd.dma_start(aux_t[:], auxd)
            wc_t = aux_t[:, 0:4].bitcast(tdt)       # [P, 4/tdt-size], cols 0:2 used
            consts_t = aux_t[:, 4:12].bitcast(f32)  # [P, 2]

            # Dummy sigmoid so the activation table load is scheduled early,
            # overlapping the table DMA instead of sitting on the tail.
            scr = sb.tile([P, 1], f32)
            nc.vector.memset(scr[:], 0.0)
            scr2 = sb.tile([P, 1], f32)
            nc.scalar.activation(scr2[:], scr[:], mybir.ActivationFunctionType.Sigmoid)

            # Writeback indices for the prepared kv_writeback (all zeros).
            idxs = sb.tile([P, 1], i32)
            nc.vector.memset(idxs[:], 0)

            # Seed fo with an early producer so the kv_writeback prep (whose
            # src read really happens at trigger time) can schedule its
            # descriptor generation early, off the critical path.  The real
            # data dependency is carried by the trigger via signals_writable.
            fo = sb.tile([P, NBLK], f32)
            nc.vector.memset(fo[:], 0.0)
            out4 = out.rearrange("(b dhi dho n) -> b dhi dho n", b=1, dhi=P, dho=1)
            fo4 = fo[:].rearrange("p (dho b n) -> p dho b n", dho=1, b=1)
            wb_sem = nc.alloc_semaphore("wb_dma")
            nc.gpsimd.kv_writeback(
                out4, fo4, idxs[:], prepare_only=True, sem=wb_sem
            )

            et = sb.tile([P, N], tdt)
            step = N // DMA_CHUNKS
            for c in range(DMA_CHUNKS):
                nc.sync.dma_start(
                    et[:, c * step : (c + 1) * step],
                    eTd[:, c * step : (c + 1) * step],
                )

            # pt[p, 2b + t]: t=0 -> a_{128b+p} (w_l GEMV), t=1 -> col_{128b+p}
            pt = ps.tile([P, 2 * NBLK], f32)
            pt3 = pt[:].rearrange("p (n two) -> p n two", two=2)
            a_col = pt[:, 1:2]
            for b in range(NBLK):
                nc.tensor.matmul(
                    pt3[:, b, :],
                    et[:, b * P : (b + 1) * P],
                    wc_t[:, 0:2],
                    start=True,
                    stop=True,
                )

            colv = bass.AP(a_col.tensor, a_col.offset, [a_col.ap[0], [2, NBLK]])
            colm = sb.tile([P, 1], f32)
            nc.vector.reduce_max(colm[:], colv, axis=mybir.AxisListType.X)
            colmax = sb.tile([P, 1], f32)
            nc.gpsimd.partition_all_reduce(
                colmax[:], colm[:], channels=P, reduce_op=bass_isa.ReduceOp.max
            )

            # k1 = colmax/s + (rw + b);  k2 = sigmoid(colmax/s + (cw + b))
            k1 = sb.tile([P, 1], f32)
            nc.vector.tensor_scalar(
                k1[:], colmax[:], inv_s, consts_t[:, 0:1],
                op0=mybir.AluOpType.mult, op1=mybir.AluOpType.add,
            )
            k2 = sb.tile([P, 1], f32)
            nc.scalar.activation(
                k2[:], colmax[:], mybir.ActivationFunctionType.Sigmoid,
                bias=consts_t[:, 1:2], scale=inv_s,
            )

            # out = sigmoid(a/s + k1) * k2
            a_row = pt[:, 0:1]
            rowv = bass.AP(a_row.tensor, a_row.offset, [a_row.ap[0], [2, NBLK]])
            so = sb.tile([P, NBLK], f32)
            nc.scalar.activation(
                so[:], rowv, mybir.ActivationFunctionType.Sigmoid,
                bias=k1[:, 0:1], scale=inv_s,
            )
            nc.vector.tensor_scalar_mul(fo[:], so[:], k2[:, 0:1])

            # Fire the prepared writeback.  signals_writable puts a WAW edge
            # on fo so the trigger (Pool, in-order) waits for the real fo
            # write; only trigger+transfer+sem sit on the tail.
            nc.gpsimd.trigger_dma(count=None, signals_writable=[fo[:]])

    nc.compile()
    return nc


def patch_for_timeline_sim(nc):
    """Make the module simulable by the no_exec TimelineSim.

    Tile schedules the kv_writeback prep on a DMASW proc lane and the final
    drain waits on that lane's semaphore.  CoreSim and real hardware satisfy
    it through their internal SWDGE ring bookkeeping, but the no_exec
    TimelineSim only fires on_update semaphores, so the wait starves.
    Attach the lane increment to the explicit wait_ge(wb_sem) instruction:
    it only becomes runnable after the actual DMA-completion semaphore, so
    the modeled timing stays honest.  Call this only on a module that is
    done running on hardware/CoreSim.
    """
    import concourse.mybir as mybir

    fn = nc.m.functions[0]
    insts = [i for blk in fn.blocks for i in blk.instructions]
    dmasw = {}
    for inst in insts:
        si = inst.sync_info
        if si is None:
            continue
        for w in si.on_wait or []:
            if (w.ant_name or "").startswith("DMASW"):
                dmasw[w.id] = w.ant_name
    for inst in insts:
        if getattr(inst, "op_name", None) != "InstIncSwdgeSem":
            continue
        vec = list(inst.instr)
        hit = [(i, v) for i, v in enumerate(vec) if v in dmasw]
        if not hit:
            continue
        idx, sid = hit[0]
        amount = next((v for v in vec[idx + 1 :] if v > 0), 16)
        si = inst.sync_info
        si.on_update = list(si.on_update or []) + [
            mybir.SyncUpdate(
                sync_type="semaphore", id=sid, ant_name=dmasw[sid],
                update_mode="sem-add-imm", update_value=amount,
                update_reg=None,
            )
        ]
    return nc


def get_nc(repeat=1, col_dt=COL_DT):
    key = ("nc", repeat, col_dt)
    if key not in _CACHE:
        _CACHE[key] = _build_nc(repeat, col_dt)
    return _CACHE[key]


def prepare_in_maps(
    anon_e_emb, e_table, c_table, r_table, fc0_w, fc0_b, c_id, r_id, col_dt=COL_DT
):
    import ml_dtypes

    e_all = np.concatenate(
        [np.asarray(e_table, np.float32), np.asarray(anon_e_emb, np.float32)], 0
    )
    fc0_w = np.asarray(fc0_w, np.float32)
    w_l = fc0_w[0, :D]
    w_r = fc0_w[0, D:]
    b = np.float32(np.asarray(fc0_b, np.float32)[0])
    c_emb = np.asarray(c_table, np.float32)[int(c_id)]
    r_emb = np.asarray(r_table, np.float32)[int(r_id)]
    rw = np.float32(np.dot(r_emb, w_l))
    cw = np.float32(np.dot(c_emb, w_l))

    if col_dt == "fp8":
        ndt, se, sw = ml_dtypes.float8_e3m4, SE, SW
    else:
        ndt, se, sw = np.float16, 1.0, 1.0
    eT = np.ascontiguousarray((e_all.T * se).astype(ndt))          # [128, 8192]
    wc = np.ascontiguousarray(
        (np.stack([w_l, w_r], axis=1) * sw).astype(ndt)            # [128, 2]
    )
    aux = np.zeros((P, 12), np.uint8)
    aux[:, 0:2] = wc.view(np.uint8)
    consts = np.empty((P, 2), np.float32)
    consts[:, 0] = rw + b
    consts[:, 1] = cw + b
    aux[:, 4:12] = consts.view(np.uint8)

    in_map = {"eT": eT, "aux": aux}
    return [dict(in_map) for _ in range(NCORES)]


def unscramble(out_dev: np.ndarray) -> np.ndarray:
    """Device layout [p*NBLK + n] -> true row order [n*P + p]."""
    return np.ascontiguousarray(out_dev.reshape(P, NBLK).T.reshape(-1))


def run(inputs, trace=False, trace_kwargs=None, repeat=1, col_dt=COL_DT):
    from concourse.bass_utils import run_bass_kernel_spmd

    nc = get_nc(repeat, col_dt)
    in_maps = prepare_in_maps(**inputs, col_dt=col_dt)
    res = run_bass_kernel_spmd(
        nc,
        in_maps,
        core_ids=list(range(NCORES)),
        trace=trace,
        **(trace_kwargs or {}),
    )
    out = unscramble(np.asarray(res.results[0]["out"]))
    return out, res


def kernel(**inputs) -> np.ndarray:
    out, _ = run(inputs, trace=False)
    return out
